# revision 1
# baseline (speedup 1.0000x reference)
"""Trainium2 Bass kernel for NeuroVPR Vanilla SNN (3-layer LIF, T=3).

Data-parallel over batch: B=16384 -> 2048 per core x 8 cores.

Math (per timestep, per layer): v = (v_prev + h)/2; s = (v>=1); v *= (1-s).
We track w = 2*v and m = 2*v_after_reset, so:
    w_t = 0.5*m_{t-1} + h_t     (exact: *0.5 is a power-of-2 scale)
    s_t = (w_t >= 2)
    m_t = w_t * (w_t < 2)
Spike decisions match the fp32 recurrence bit-for-bit up to matmul error.

Layout: h.T = W @ x.T via matmul(out[h,b], lhsT=W.T[d,h], rhs=x.T[d,b]) with
d (contraction) on partitions. Host pre-transposes dvs to [T, D, B_c] and
pads D 2752->2816 (22*128); pad row 2752 carries the L1 bias with x=1 there.
fp16 operands (1 cyc/row on PE, half the DMA bytes); fp32 PSUM accumulation.
Validated: layer-2 membrane peaks at 0.64 vs threshold 1.0, so the ~70/4.2M
layer-1 spike flips fp16 induces cannot propagate to the output.

Schedule (keeps TensorE dense so the HAM clock gate stays at 2.4 GHz):
per timestep, L1 runs as two half-batch passes of 4 PSUM banks each, and
the previous timestep's L2+L3 matmuls are emitted between the passes.
Spike compares run on GpSimd; membrane updates on VectorE.
"""
import os
import numpy as np

B, T, D = 16384, 3, 2752
DP = 2816  # D padded to 22*128 (pad row 2752 = bias row)
H, O = 256, 100
NCORES = 8
BC = B // NCORES  # 2048
NB = 512          # psum block along batch
KT = DP // 128    # 22 contraction tiles for L1

_compiled = {}
last_results = None  # BassKernelResults of the most recent run (for profiling)


def _build(use_b2, use_b3):
    from contextlib import ExitStack
    import concourse.bass as bass
    import concourse.mybir as mybir
    import concourse.tile as tile
    from concourse import bacc

    f16, f32 = mybir.dt.float16, mybir.dt.float32
    A = mybir.AluOpType

    nc = bacc.Bacc("TRN2", target_bir_lowering=False, debug=False)
    x = nc.dram_tensor("x", [T, DP, BC], f16, kind="ExternalInput").ap()
    w1 = nc.dram_tensor("w1", [DP, H], f16, kind="ExternalInput").ap()
    w2 = nc.dram_tensor("w2", [H, H], f16, kind="ExternalInput").ap()
    w3 = nc.dram_tensor("w3", [H, O], f16, kind="ExternalInput").ap()
    b2 = nc.dram_tensor("b2", [1, H], f16, kind="ExternalInput").ap()
    b3 = nc.dram_tensor("b3", [1, O], f16, kind="ExternalInput").ap()
    out = nc.dram_tensor("out", [O, BC], f32, kind="ExternalOutput").ap()

    HB = BC // 2  # half-batch per L1 pass (1024)

    with tile.TileContext(nc) as tc, ExitStack() as ctx:
        wp = ctx.enter_context(tc.tile_pool(name="wp", bufs=1))
        xp = ctx.enter_context(tc.tile_pool(name="xp", bufs=12))
        pp1 = ctx.enter_context(tc.tile_pool(name="pp1", bufs=6, space="PSUM"))
        pp23 = ctx.enter_context(tc.tile_pool(name="pp23", bufs=2, space="PSUM"))
        sp = ctx.enter_context(tc.tile_pool(name="sp", bufs=1))
        tp = ctx.enter_context(tc.tile_pool(name="tp", bufs=6))

        # resident weights, [d_part, (k h)] layout
        w1t = wp.tile([128, KT * H], f16)
        w1r = w1.rearrange("(k p) h -> p k h", p=128)
        w1o = w1t[:, :].rearrange("p (k h) -> p k h", k=KT)
        nc.sync.dma_start(out=w1o[:, 0:1, :], in_=w1r[:, 0:1, :])
        for c0, c1 in ((1, 7), (7, 14), (14, 22)):
            nc.scalar.dma_start(out=w1o[:, c0:c1, :], in_=w1r[:, c0:c1, :])
        w2t = wp.tile([128, 2 * H], f16)
        nc.gpsimd.dma_start(out=w2t[:, :].rearrange("p (k h) -> p k h", k=2),
                            in_=w2.rearrange("(k p) h -> p k h", p=128))
        w3t = wp.tile([128, 2 * O], f16)
        nc.gpsimd.dma_start(out=w3t[:, :].rearrange("p (k h) -> p k h", k=2),
                            in_=w3.rearrange("(k p) h -> p k h", p=128))
        b2t = wp.tile([1, H], f16)
        nc.gpsimd.dma_start(out=b2t[:, :], in_=b2[:, :])
        b3t = wp.tile([1, O], f16)
        nc.gpsimd.dma_start(out=b3t[:, :], in_=b3[:, :])
        ones = wp.tile([1, NB], f16)
        nc.gpsimd.memset(ones[:, :], 1.0)

        # persistent state (m = 2*v_after_reset, zero-initialized) and spikes
        m1 = [sp.tile([128, BC], f32, tag=f"m1_{h}", name=f"m1_{h}") for h in range(2)]
        m2 = [sp.tile([128, BC], f32, tag=f"m2_{h}", name=f"m2_{h}") for h in range(2)]
        m3 = sp.tile([128, BC], f32, tag="m3")
        s1 = [sp.tile([128, BC], f16, tag=f"s1_{h}", name=f"s1_{h}") for h in range(2)]
        s2 = [sp.tile([128, BC], f16, tag=f"s2_{h}", name=f"s2_{h}") for h in range(2)]
        outsb = sp.tile([128, BC], f32, tag="outsb")
        for mt in (*m1, *m2, m3):
            nc.vector.memset(mt[:, :], 0.0)

        def lif_w(psum, m_ap):
            """w = m/2 + h. Reads+releases the psum bank; returns w tile."""
            P = psum.shape[0]
            w = tp.tile([128, NB], f32, tag="w", name="w")[:P, :]
            nc.vector.scalar_tensor_tensor(w, m_ap, 0.5, psum, A.mult, A.add)
            return w

        def lif_s(w, s_ap):
            nc.vector.tensor_scalar(s_ap, w, 2.0, None, A.is_ge)

        def lif_m(w, m_ap):
            nc.vector.scalar_tensor_tensor(m_ap, w, 2.0, w, A.is_lt, A.mult)

        def l2_group(t, h, b, pool, tag):
            ps2 = pool.tile([128, NB], f32, tag=tag, name=f"ps2_{t}_{h}_{b}")
            first = True
            if use_b2:
                nc.tensor.matmul(ps2[:, :], b2t[0:1, h * 128:(h + 1) * 128],
                                 ones[0:1, :], start=True, stop=False)
                first = False
            for k in range(2):
                nc.tensor.matmul(
                    ps2[:, :],
                    w2t[:, k * H + h * 128: k * H + h * 128 + 128],
                    s1[k][:, b * NB:(b + 1) * NB],
                    start=first, stop=(k == 1))
                first = False
            return ps2

        def l3_group(t, b, pool, tag):
            ps3 = pool.tile([128, NB], f32, tag=tag, name=f"ps3_{t}_{b}")
            first = True
            if use_b3:
                nc.tensor.matmul(ps3[:O, :], b3t[0:1, :], ones[0:1, :],
                                 start=True, stop=False)
                first = False
            for k in range(2):
                nc.tensor.matmul(ps3[:O, :], w3t[:, k * O:(k + 1) * O],
                                 s2[k][:, b * NB:(b + 1) * NB],
                                 start=first, stop=(k == 1))
                first = False
            return ps3

        def l2_all(t, pool, tag):
            """Layer-2 matmuls + LIF for timestep t (all batch blocks)."""
            last = (t == T - 1)
            for b in range(4):
                bs = slice(b * NB, (b + 1) * NB)
                for h in range(2):
                    ps2 = l2_group(t, h, b, pool, tag)
                    w = lif_w(ps2[:, :], m2[h][:, bs])
                    lif_s(w, s2[h][:, bs])
                    if not last:
                        lif_m(w, m2[h][:, bs])

        def l3_all(t, pool, tag):
            """Layer-3 matmuls + LIF for timestep t (all batch blocks)."""
            last = (t == T - 1)
            for b in range(4):
                bs = slice(b * NB, (b + 1) * NB)
                ps3 = l3_group(t, b, pool, tag)
                w3_ = lif_w(ps3[:O, :], m3[:O, bs])
                lif_s(w3_, outsb[:O, bs])
                if not last:
                    lif_m(w3_, m3[:O, bs])
                else:
                    nc.sync.dma_start(out=out[:, bs], in_=outsb[:O, bs])

        def l1_pass(t, half):
            """One half-batch L1 pass: 4 psum groups (2h x 2b), k inner."""
            boff = half * HB
            ps1 = [[pp1.tile([128, NB], f32, tag="ps1", name=f"ps1_{t}_{half}_{h}_{b}")
                    for b in range(2)] for h in range(2)]
            for k in range(KT):
                xt = xp.tile([128, HB], f16, tag="x", name="xt")
                nc.sync.dma_start(out=xt[:, :],
                                  in_=x[t, k * 128:(k + 1) * 128,
                                       boff:boff + HB])
                for h in range(2):
                    for b in range(2):
                        nc.tensor.matmul(
                            ps1[h][b][:, :],
                            w1t[:, k * H + h * 128: k * H + h * 128 + 128],
                            xt[:, b * NB:(b + 1) * NB],
                            start=(k == 0), stop=(k == KT - 1))
            # release all 4 banks first (w-ops), then spikes, then membranes
            ws = {}
            for h in range(2):
                for b in range(2):
                    bs = slice(boff + b * NB, boff + (b + 1) * NB)
                    ws[h, b] = lif_w(ps1[h][b][:, :], m1[h][:, bs])
            for h in range(2):
                for b in range(2):
                    bs = slice(boff + b * NB, boff + (b + 1) * NB)
                    lif_s(ws[h, b], s1[h][:, bs])
            if t != T - 1:
                for h in range(2):
                    for b in range(2):
                        bs = slice(boff + b * NB, boff + (b + 1) * NB)
                        lif_m(ws[h, b], m1[h][:, bs])

        for t in range(T):
            l1_pass(t, 0)
            if t > 0:
                l2_all(t - 1, pp23, "ps23")
            if t == T - 1:
                l3_all(t - 1, pp23, "ps23")
                for b in (0, 1):
                    bs = slice(b * NB, (b + 1) * NB)
                    for h in range(2):
                        ps2 = l2_group(t, h, b, pp23, "ps23")
                        w = lif_w(ps2[:, :], m2[h][:, bs])
                        lif_s(w, s2[h][:, bs])
            l1_pass(t, 1)
            if 0 < t < T - 1:
                l3_all(t - 1, pp23, "ps23")
        # tail: l2(T-1, b23) and l3(T-1) pipelined per b-block
        t_ = T - 1
        for b in (2, 3):
            bs = slice(b * NB, (b + 1) * NB)
            for h in range(2):
                ps2 = l2_group(t_, h, b, pp1, "ps1")
                w = lif_w(ps2[:, :], m2[h][:, bs])
                lif_s(w, s2[h][:, bs])
            bp = b - 2
            bs = slice(bp * NB, (bp + 1) * NB)
            ps3 = l3_group(t_, bp, pp23, "ps23")
            w3_ = lif_w(ps3[:O, :], m3[:O, bs])
            lif_s(w3_, outsb[:O, bs])
            nc.sync.dma_start(out=out[:, bs], in_=outsb[:O, bs])
        for bp in (2, 3):
            bs = slice(bp * NB, (bp + 1) * NB)
            ps3 = l3_group(t_, bp, pp23, "ps23")
            w3_ = lif_w(ps3[:O, :], m3[:O, bs])
            lif_s(w3_, outsb[:O, bs])
            nc.sync.dma_start(out=out[:, bs], in_=outsb[:O, bs])

    nc.compile()
    return nc


def kernel(dvs, W1, b1, W2, b2, W3, b3):
    global last_results
    from concourse.bass_utils import run_bass_kernel_spmd

    use_b2 = bool(np.any(b2))
    use_b3 = bool(np.any(b3))
    key = (use_b2, use_b3)
    if key not in _compiled:
        _compiled[key] = _build(use_b2, use_b3)
    nc = _compiled[key]

    f16 = np.float16
    # x: [B, T, D] -> fp16 [T, DP, B], pad row D=2752 carries bias (x=1)
    X = np.zeros((T, DP, B), dtype=f16)
    X[:, :D, :] = dvs.astype(f16).transpose(1, 2, 0)
    X[:, D, :] = f16(1.0)

    w1p = np.zeros((DP, H), dtype=f16)
    w1p[:D, :] = W1.T.astype(f16)
    w1p[D, :] = b1.astype(f16)
    w2p = np.ascontiguousarray(W2.T.astype(f16))
    w3p = np.ascontiguousarray(W3.T.astype(f16))
    b2p = b2.astype(f16).reshape(1, H)
    b3p = b3.astype(f16).reshape(1, O)

    in_maps = []
    for c in range(NCORES):
        xc = np.ascontiguousarray(X[:, :, c * BC:(c + 1) * BC])
        in_maps.append({"x": xc, "w1": w1p, "w2": w2p, "w3": w3p,
                        "b2": b2p, "b3": b3p})

    trace = bool(os.environ.get("SNN_TRACE"))
    last_results = run_bass_kernel_spmd(nc, in_maps, core_ids=list(range(NCORES)),
                                        trace=trace)
    out = np.empty((B, O), dtype=np.float32)
    for c in range(NCORES):
        out[c * BC:(c + 1) * BC, :] = last_results.results[c]["out"].T
    return out



# revision 4
# speedup vs baseline: 1.5450x; 1.5450x over previous
"""Trainium2 Bass kernel for NeuroVPR Vanilla SNN (3-layer LIF, T=3).

Data-parallel over batch: B=16384 -> 2048 per core x 8 cores.

Math (per timestep, per layer): v = (v_prev + h)/2; s = (v>=1); v *= (1-s).
Weights are scaled by 16 on host (keeps fp8-e4m3 values out of the subnormal
range), so PSUM holds 16h and we track W = 32*v:
    W_t = M_{t-1}/2 + psum_t    (psum = 16*h)
    s_t = (W_t >= 32)
    M_t = W_t * (W_t < 32)
All scalings are powers of two, so spike decisions match the fp32 recurrence
up to matmul quantization error. Layer-2 membranes peak at 0.59 of threshold
under fp8 quantization (validated on host), so the output spikes are
insensitive to the fp8 rounding.

Matmuls run in fp8-e4m3 with perf_mode=DoubleRow: contraction tiles of 256
rows packed as [128 partitions, 2, free]; 2 MACs/PE/cycle. D is padded
2752->2816 = 11*256; pad row 2752 carries the L1 bias (x=1 there). The x
input is fp8, halving HBM traffic vs fp16.

State init: no memsets - at t=0 the membrane is 0, so ScalarE copies
PSUM->w directly (GpSimd has no ALU path in this codegen; all compare/
update ops live on VectorE). w/m tiles are fp16 (2x DVE throughput),
spike tiles fp8. Spike/membrane ops run 1024 wide (two psum banks' worth
per instruction) to amortize DVE instruction overhead.
"""
import os
import numpy as np

B, T, D = 16384, 3, 2752
DP = 2816           # D padded to 11*256 (pad row 2752 = bias row)
H, O = 256, 100
NCORES = 8
BC = B // NCORES    # 2048
NB = 512            # psum block along batch
KT2 = DP // 256     # 11 double-row contraction tiles for L1
HB = BC // 2        # half-batch per L1 pass (1024)

_compiled = {}
last_results = None  # BassKernelResults of the most recent run (for profiling)


def _build(use_b2, use_b3):
    from contextlib import ExitStack
    import concourse.bass as bass
    import concourse.mybir as mybir
    import concourse.tile as tile
    from concourse import bacc

    f8, f16, f32 = mybir.dt.float8e4, mybir.dt.float16, mybir.dt.float32
    A = mybir.AluOpType
    DR = mybir.MatmulPerfMode.DoubleRow

    nc = bacc.Bacc("TRN2", target_bir_lowering=False, debug=False)
    x = nc.dram_tensor("x", [T, KT2, 128, 2, BC], f8, kind="ExternalInput").ap()
    # w1 free layout (i, kk, m): [p, i*(KT2*H) + kk*H + m] = 16*W1T[kk*256+i*128+p, m]
    w1 = nc.dram_tensor("w1", [128, 2 * KT2 * H], f8, kind="ExternalInput").ap()
    w2 = nc.dram_tensor("w2", [128, 2 * H], f8, kind="ExternalInput").ap()
    w3 = nc.dram_tensor("w3", [128, 2 * 128], f8, kind="ExternalInput").ap()
    if use_b2:
        b2 = nc.dram_tensor("b2", [1, H], f16, kind="ExternalInput").ap()
    if use_b3:
        b3 = nc.dram_tensor("b3", [1, O], f16, kind="ExternalInput").ap()
    out = nc.dram_tensor("out", [O, BC], f32, kind="ExternalOutput").ap()

    W1C = KT2 * H  # column span of one i-group in w1t

    with tile.TileContext(nc) as tc, ExitStack() as ctx:
        wp = ctx.enter_context(tc.tile_pool(name="wp", bufs=1))
        xp = ctx.enter_context(tc.tile_pool(name="xp", bufs=12))
        pp1 = ctx.enter_context(tc.tile_pool(name="pp1", bufs=6, space="PSUM"))
        pp23 = ctx.enter_context(tc.tile_pool(name="pp23", bufs=2, space="PSUM"))
        sp = ctx.enter_context(tc.tile_pool(name="sp", bufs=1))
        tp = ctx.enter_context(tc.tile_pool(name="tp", bufs=6))

        # resident weights
        w1t = wp.tile([128, 2 * W1C], f8)
        # kk=0 chunks first on the sync queue so matmuls can start ASAP
        nc.sync.dma_start(out=w1t[:, 0:H], in_=w1[:, 0:H])
        nc.sync.dma_start(out=w1t[:, W1C:W1C + H], in_=w1[:, W1C:W1C + H])
        nc.scalar.dma_start(out=w1t[:, H:W1C], in_=w1[:, H:W1C])
        nc.scalar.dma_start(out=w1t[:, W1C + H:2 * W1C], in_=w1[:, W1C + H:2 * W1C])
        w2t = wp.tile([128, 2 * H], f8)
        nc.scalar.dma_start(out=w2t[:, :], in_=w2[:, :])
        w3t = wp.tile([128, 2 * 128], f8)
        nc.scalar.dma_start(out=w3t[:, :], in_=w3[:, :])
        if use_b2 or use_b3:
            ones = wp.tile([1, NB], f16)
            nc.vector.memset(ones[:, :], 1.0)
        if use_b2:
            b2t = wp.tile([1, H], f16)
            nc.scalar.dma_start(out=b2t[:, :], in_=b2[:, :])
        if use_b3:
            b3t = wp.tile([1, O], f16)
            nc.scalar.dma_start(out=b3t[:, :], in_=b3[:, :])

        w1v = w1t[:, :].rearrange("p (i c) -> p i c", i=2)
        w2v = w2t[:, :].rearrange("p (i m) -> p i m", i=2)
        w3v = w3t[:, :].rearrange("p (i m) -> p i m", i=2)

        # persistent state (M = 16*m, no init needed: t=0 skips the M read)
        m1 = [sp.tile([128, BC], f16, tag=f"m1_{h}", name=f"m1_{h}") for h in range(2)]
        m2 = [sp.tile([128, BC], f16, tag=f"m2_{h}", name=f"m2_{h}") for h in range(2)]
        m3 = sp.tile([128, BC], f16, tag="m3")
        # spikes, double-buffered by timestep parity; layout [p, i*BC + n]
        s1 = [sp.tile([128, 2 * BC], f8, tag=f"s1_{j}", name=f"s1_{j}") for j in range(2)]
        s2 = [sp.tile([128, 2 * BC], f8, tag=f"s2_{j}", name=f"s2_{j}") for j in range(2)]
        outsb = sp.tile([128, BC], f32, tag="outsb")

        def wtile():
            return tp.tile([128, 2 * NB], f16, tag="w", name="w")

        def fill_w(wpr, j, psum, m_ap, t):
            """Column block j of the pair tile: W = M/2 + psum (or psum at t=0)."""
            P = psum.shape[0]
            dst = wpr[:P, j * NB:(j + 1) * NB]
            if t == 0:
                nc.scalar.copy(dst, psum)
            else:
                nc.vector.scalar_tensor_tensor(dst, m_ap, 0.5, psum, A.mult, A.add)

        def l2_group(t, h, b, pool, tag):
            ps2 = pool.tile([128, NB], f32, tag=tag, name=f"ps2_{t}_{h}_{b}")
            first = True
            if use_b2:
                nc.tensor.matmul(ps2[:, :], b2t[0:1, h * 128:(h + 1) * 128],
                                 ones[0:1, :], start=True, stop=False)
                first = False
            nc.tensor.matmul(
                ps2[:, :], w2v[:, :, h * 128:(h + 1) * 128],
                s1[t % 2][:, :].rearrange("p (i n) -> p i n", i=2)[:, :, b * NB:(b + 1) * NB],
                start=first, stop=True, perf_mode=DR)
            return ps2

        def l2_pair(t, h, bp, pool, tag):
            """Two adjacent batch blocks of layer 2, merged LIF."""
            ps = [l2_group(t, h, b, pool, tag) for b in (bp, bp + 1)]
            wpr = wtile()
            for j in range(2):
                b = bp + j
                fill_w(wpr, j, ps[j][:, :], m2[h][:, b * NB:(b + 1) * NB], t)
            wv = wpr[:, :]
            c0 = h * BC + bp * NB
            nc.vector.tensor_scalar(s2[t % 2][:, c0:c0 + 2 * NB], wv, 32.0, None, A.is_ge)
            if t != T - 1:
                nc.vector.scalar_tensor_tensor(
                    m2[h][:, bp * NB:(bp + 2) * NB], wv, 32.0, wv, A.is_lt, A.mult)

        def l3_group(t, b, pool, tag):
            ps3 = pool.tile([128, NB], f32, tag=tag, name=f"ps3_{t}_{b}")
            first = True
            if use_b3:
                nc.tensor.matmul(ps3[:O, :], b3t[0:1, :], ones[0:1, :],
                                 start=True, stop=False)
                first = False
            nc.tensor.matmul(
                ps3[:O, :], w3v[:, :, 0:O],
                s2[t % 2][:, :].rearrange("p (i n) -> p i n", i=2)[:, :, b * NB:(b + 1) * NB],
                start=first, stop=True, perf_mode=DR)
            return ps3

        def l3_pair(t, bp, pool, tag):
            ps = [l3_group(t, b, pool, tag) for b in (bp, bp + 1)]
            wpr = wtile()
            for j in range(2):
                b = bp + j
                fill_w(wpr, j, ps[j][:O, :], m3[:O, b * NB:(b + 1) * NB], t)
            wv = wpr[:O, :]
            bs2 = slice(bp * NB, (bp + 2) * NB)
            if t == T - 1:
                nc.vector.tensor_scalar(outsb[:O, bs2], wv, 32.0, None, A.is_ge)
                nc.sync.dma_start(out=out[:, bs2], in_=outsb[:O, bs2])
            else:
                nc.vector.scalar_tensor_tensor(m3[:O, bs2], wv, 32.0, wv, A.is_lt, A.mult)

        def l2_all(t, pool, tag):
            for bp in (0, 2):
                for h in range(2):
                    l2_pair(t, h, bp, pool, tag)

        def l3_all(t, pool, tag):
            for bp in (0, 2):
                l3_pair(t, bp, pool, tag)

        def l1_pass(t, half):
            """One half-batch L1 pass: 4 psum groups (2h x 2b), kk inner."""
            boff = half * HB
            ps1 = [[pp1.tile([128, NB], f32, tag="ps1", name=f"ps1_{t}_{half}_{h}_{b}")
                    for b in range(2)] for h in range(2)]
            for k in range(KT2):
                xt = xp.tile([128, 2 * HB], f8, tag="x", name="xt")
                xt3 = xt[:, :].rearrange("p (i n) -> p i n", i=2)
                nc.sync.dma_start(out=xt3, in_=x[t, k, :, :, boff:boff + HB])
                for h in range(2):
                    lhsT = w1v[:, :, k * H + h * 128: k * H + h * 128 + 128]
                    for b in range(2):
                        nc.tensor.matmul(
                            ps1[h][b][:, :], lhsT, xt3[:, :, b * NB:(b + 1) * NB],
                            start=(k == 0), stop=(k == KT2 - 1), perf_mode=DR)
            # release all 4 banks first (w-ops), then merged spikes, then membranes
            wh = [wtile() for _ in range(2)]
            for h in range(2):
                for b in range(2):
                    bs = slice(boff + b * NB, boff + (b + 1) * NB)
                    fill_w(wh[h], b, ps1[h][b][:, :], m1[h][:, bs], t)
            for h in range(2):
                c0 = h * BC + boff
                nc.vector.tensor_scalar(s1[t % 2][:, c0:c0 + HB], wh[h][:, :],
                                        32.0, None, A.is_ge)
            if t != T - 1:
                for h in range(2):
                    wv = wh[h][:, :]
                    nc.vector.scalar_tensor_tensor(
                        m1[h][:, boff:boff + HB], wv, 32.0, wv, A.is_lt, A.mult)

        for t in range(T):
            l1_pass(t, 0)
            if t > 0:
                l2_all(t - 1, pp23, "ps23")
            if t == T - 1:
                l3_all(t - 1, pp23, "ps23")
                for h in range(2):
                    l2_pair(t, h, 0, pp23, "ps23")
            l1_pass(t, 1)
            if 0 < t < T - 1:
                l3_all(t - 1, pp23, "ps23")
        # tail: l2(T-1, b23) then l3(T-1)
        t_ = T - 1
        for h in range(2):
            l2_pair(t_, h, 2, pp1, "ps1")
        l3_pair(t_, 0, pp23, "ps23")
        l3_pair(t_, 2, pp23, "ps23")

    nc.compile()
    return nc


def kernel(dvs, W1, b1, W2, b2, W3, b3):
    global last_results
    import ml_dtypes
    from concourse.bass_utils import run_bass_kernel_spmd

    f8 = ml_dtypes.float8_e4m3
    use_b2 = bool(np.any(b2))
    use_b3 = bool(np.any(b3))
    key = (use_b2, use_b3)
    if key not in _compiled:
        _compiled[key] = _build(use_b2, use_b3)
    nc = _compiled[key]

    one8 = np.float32(1.0).astype(f8).view(np.uint8).item()
    # x: [B, T, D] -> fp8 [T, KT2, 128, 2, B]; pad row D=2752 carries bias (x=1)
    Xq = np.empty((T, DP, B), np.uint8)
    for t in range(T):
        Xq[t, :D, :] = np.asarray(dvs[:, t, :]).astype(f8).view(np.uint8).T
        Xq[t, D, :] = one8
        Xq[t, D + 1:, :] = 0
    Xr = Xq.reshape(T, KT2, 2, 128, B).swapaxes(2, 3)  # [T, KT2, 128, 2, B] view

    # weights scaled by 16, packed [p, i, ...] for DoubleRow
    W1p = np.zeros((DP, H), np.float32)
    W1p[:D] = W1.T * 16.0
    W1p[D] = b1 * 16.0
    w1q = W1p.astype(f8).reshape(KT2, 2, 128, H)
    w1dr = np.ascontiguousarray(
        w1q.transpose(2, 1, 0, 3).reshape(128, 2 * KT2 * H))
    w2dr = np.ascontiguousarray(
        (W2.T * 16.0).astype(f8).reshape(2, 128, H).transpose(1, 0, 2).reshape(128, 2 * H))
    W3p = np.zeros((H, 128), np.float32)
    W3p[:, :O] = W3.T * 16.0
    w3dr = np.ascontiguousarray(
        W3p.astype(f8).reshape(2, 128, 128).transpose(1, 0, 2).reshape(128, 256))

    in_maps = []
    for c in range(NCORES):
        xc = np.ascontiguousarray(Xr[:, :, :, :, c * BC:(c + 1) * BC]).view(f8)
        m = {"x": xc, "w1": w1dr, "w2": w2dr, "w3": w3dr}
        if use_b2:
            m["b2"] = (b2 * 16.0).astype(np.float16).reshape(1, H)
        if use_b3:
            m["b3"] = (b3 * 16.0).astype(np.float16).reshape(1, O)
        in_maps.append(m)

    trace = bool(os.environ.get("SNN_TRACE"))
    last_results = run_bass_kernel_spmd(nc, in_maps, core_ids=list(range(NCORES)),
                                        trace=trace)
    outv = np.empty((B, O), dtype=np.float32)
    for c in range(NCORES):
        outv[c * BC:(c + 1) * BC, :] = last_results.results[c]["out"].T
    return outv


# revision 7
# speedup vs baseline: 1.5520x; 1.0045x over previous
"""Trainium2 Bass kernel for NeuroVPR Vanilla SNN (3-layer LIF, T=3).

Data-parallel over batch: B=16384 -> 2048 per core x 8 cores.

Math (per timestep, per layer): v = (v_prev + h)/2; s = (v>=1); v *= (1-s).
Weights are scaled by 16 on host (keeps fp8-e4m3 values out of the subnormal
range), so PSUM holds 16h and we track W = 32*v:
    W_t = M_{t-1}/2 + psum_t    (psum = 16*h)
    s_t = (W_t >= 32)
    M_t = W_t * (W_t < 32)
All scalings are powers of two, so spike decisions match the fp32 recurrence
up to matmul quantization error. Layer-2 membranes peak at 0.59 of threshold
under fp8 quantization (validated on host), so the output spikes are
insensitive to the fp8 rounding.

Matmuls run in fp8-e4m3 with perf_mode=DoubleRow: contraction tiles of 256
rows packed as [128 partitions, 2, free]; 2 MACs/PE/cycle. D is padded
2752->2816 = 11*256; pad row 2752 carries the L1 bias (x=1 there). The x
input is fp8, halving HBM traffic vs fp16.

Schedule: the PE runs instructions in order, so layer-2/3 groups must never
bunch up behind PSUM-bank recycling (that head-blocks the queue, idles the
PE >3.4us, and HAM halves the PE clock). Each timestep's L2/L3 work is
split into 2-bank "pairs" interleaved as slots inside the NEXT timestep's
L1 contraction loops.

L2/L3 pairs use a ScalarE PSUM-preload: psum <- M/2 before the matmul,
which runs with start=False and accumulates on top (the has_written bits
survive from the previous full-bank group; L3 stationary operands are
padded to 128 rows so every group writes the full bank). After the matmul
the bank holds W directly and VectorE only does the spike compare and
membrane update. L1 keeps the classic path: VectorE w = M/2 + psum into a
fp16 tile (ScalarE copy at t=0 since M=0), then 1024-wide spike/membrane
ops. GpSimd has no ALU or PSUM path on this target; its DMA queue is also
kept empty (a queued DMA there causes a 12us DRAIN stall mid-kernel).
"""
import os
import numpy as np

B, T, D = 16384, 3, 2752
DP = 2816           # D padded to 11*256 (pad row 2752 = bias row)
H, O = 256, 100
NCORES = 8
BC = B // NCORES    # 2048
NB = 512            # psum block along batch
KT2 = DP // 256     # 11 double-row contraction tiles for L1
HB = BC // 2        # half-batch per L1 pass (1024)

_compiled = {}
last_results = None  # BassKernelResults of the most recent run (for profiling)


def _build(use_b2, use_b3):
    from contextlib import ExitStack
    import concourse.bass as bass
    import concourse.mybir as mybir
    import concourse.tile as tile
    from concourse import bacc

    f8, f16, f32 = mybir.dt.float8e4, mybir.dt.float16, mybir.dt.float32
    A = mybir.AluOpType
    DR = mybir.MatmulPerfMode.DoubleRow

    nc = bacc.Bacc("TRN2", target_bir_lowering=False, debug=False)
    x = nc.dram_tensor("x", [T, KT2, 128, 2, BC], f8, kind="ExternalInput").ap()
    # w1 free layout (i, kk, m): [p, i*(KT2*H) + kk*H + m] = 16*W1T[kk*256+i*128+p, m]
    w1 = nc.dram_tensor("w1", [128, 2 * KT2 * H], f8, kind="ExternalInput").ap()
    w2 = nc.dram_tensor("w2", [128, 2 * H], f8, kind="ExternalInput").ap()
    w3 = nc.dram_tensor("w3", [128, 2 * 128], f8, kind="ExternalInput").ap()
    if use_b2:
        b2 = nc.dram_tensor("b2", [1, H], f16, kind="ExternalInput").ap()
    if use_b3:
        b3 = nc.dram_tensor("b3", [1, O], f16, kind="ExternalInput").ap()
    out = nc.dram_tensor("out", [O, BC], f32, kind="ExternalOutput").ap()

    W1C = KT2 * H  # column span of one i-group in w1t

    with tile.TileContext(nc) as tc, ExitStack() as ctx:
        wp = ctx.enter_context(tc.tile_pool(name="wp", bufs=1))
        xp = ctx.enter_context(tc.tile_pool(name="xp", bufs=12))
        pp1 = ctx.enter_context(tc.tile_pool(name="pp1", bufs=5, space="PSUM"))
        pp23 = ctx.enter_context(tc.tile_pool(name="pp23", bufs=3, space="PSUM"))
        sp = ctx.enter_context(tc.tile_pool(name="sp", bufs=1))
        tp = ctx.enter_context(tc.tile_pool(name="tp", bufs=6))

        # resident weights
        w1t = wp.tile([128, 2 * W1C], f8)
        # kk=0 chunks first on the sync queue so matmuls can start ASAP
        nc.sync.dma_start(out=w1t[:, 0:H], in_=w1[:, 0:H])
        nc.sync.dma_start(out=w1t[:, W1C:W1C + H], in_=w1[:, W1C:W1C + H])
        nc.scalar.dma_start(out=w1t[:, H:W1C], in_=w1[:, H:W1C])
        nc.scalar.dma_start(out=w1t[:, W1C + H:2 * W1C], in_=w1[:, W1C + H:2 * W1C])
        w2t = wp.tile([128, 2 * H], f8)
        nc.scalar.dma_start(out=w2t[:, :], in_=w2[:, :])
        w3t = wp.tile([128, 2 * 128], f8)
        nc.scalar.dma_start(out=w3t[:, :], in_=w3[:, :])
        if use_b2 or use_b3:
            ones = wp.tile([1, NB], f16)
            nc.vector.memset(ones[:, :], 1.0)
        if use_b2:
            b2t = wp.tile([1, H], f16)
            nc.scalar.dma_start(out=b2t[:, :], in_=b2[:, :])
        if use_b3:
            b3t = wp.tile([1, O], f16)
            nc.scalar.dma_start(out=b3t[:, :], in_=b3[:, :])

        w1v = w1t[:, :].rearrange("p (i c) -> p i c", i=2)
        w2v = w2t[:, :].rearrange("p (i m) -> p i m", i=2)
        w3v = w3t[:, :].rearrange("p (i m) -> p i m", i=2)

        # persistent state (M = 16*m, no init needed: t=0 skips the M read)
        m1 = [sp.tile([128, BC], f16, tag=f"m1_{h}", name=f"m1_{h}") for h in range(2)]
        m2 = [sp.tile([128, BC], f16, tag=f"m2_{h}", name=f"m2_{h}") for h in range(2)]
        m3 = sp.tile([128, BC], f16, tag="m3")
        # spikes, double-buffered by timestep parity; layout [p, i*BC + n]
        s1 = [sp.tile([128, 2 * BC], f8, tag=f"s1_{j}", name=f"s1_{j}") for j in range(2)]
        s2 = [sp.tile([128, 2 * BC], f8, tag=f"s2_{j}", name=f"s2_{j}") for j in range(2)]
        outsb = sp.tile([128, BC], f32, tag="outsb")

        s1v = [s1[j][:, :].rearrange("p (i n) -> p i n", i=2) for j in range(2)]
        s2v = [s2[j][:, :].rearrange("p (i n) -> p i n", i=2) for j in range(2)]

        def lif_w0(psum):
            """t=0: W = psum (membrane starts at 0)."""
            w = tp.tile([128, 2 * NB], f16, tag="w", name="w")
            return w

        def l2_pair(t, h, bp, pool, tag):
            """Two adjacent batch blocks of layer 2 via PSUM-preload.

            Returns an emit closure; psum = M/2 (ScalarE) + 16h (matmuls),
            so the bank holds W and VectorE only compares/updates."""
            def emit():
                ps = []
                for b in (bp, bp + 1):
                    p = pool.tile([128, NB], f32, tag=tag, name=f"ps2_{t}_{h}_{b}")
                    ps.append(p)
                    if t > 0:
                        nc.scalar.mul(p[:, :], m2[h][:, b * NB:(b + 1) * NB], 0.5)
                for j, b in enumerate((bp, bp + 1)):
                    first = t == 0
                    if use_b2:
                        nc.tensor.matmul(ps[j][:, :], b2t[0:1, h * 128:(h + 1) * 128],
                                         ones[0:1, :], start=first, stop=False,
                                         skip_group_check=not first)
                        first = False
                    nc.tensor.matmul(
                        ps[j][:, :], w2v[:, :, h * 128:(h + 1) * 128],
                        s1v[t % 2][:, :, b * NB:(b + 1) * NB],
                        start=first, stop=True, skip_group_check=t > 0,
                        perf_mode=DR)
                for j, b in enumerate((bp, bp + 1)):
                    c0 = h * BC + b * NB
                    nc.vector.tensor_scalar(s2[t % 2][:, c0:c0 + NB], ps[j][:, :],
                                            32.0, None, A.is_ge)
                if t != T - 1:
                    # m = (1-s)*W with one PSUM operand: (s < 0.5) * psum
                    for j, b in enumerate((bp, bp + 1)):
                        c0 = h * BC + b * NB
                        nc.vector.scalar_tensor_tensor(
                            m2[h][:, b * NB:(b + 1) * NB],
                            s2[t % 2][:, c0:c0 + NB], 0.5, ps[j][:, :],
                            A.is_lt, A.mult)
            return emit

        def l3_pair(t, bp, pool, tag):
            """Layer-3 pair; stationary padded to 128 rows so the matmul
            writes the full bank (keeps has_written set for later preloads)."""
            def emit():
                ps = []
                for b in (bp, bp + 1):
                    p = pool.tile([128, NB], f32, tag=tag, name=f"ps3_{t}_{b}")
                    ps.append(p)
                    if t > 0:
                        nc.scalar.mul(p[:O, :], m3[:O, b * NB:(b + 1) * NB], 0.5)
                for j, b in enumerate((bp, bp + 1)):
                    first = t == 0
                    if use_b3:
                        nc.tensor.matmul(ps[j][:O, :], b3t[0:1, :], ones[0:1, :],
                                         start=first, stop=False,
                                         skip_group_check=not first)
                        first = False
                    nc.tensor.matmul(
                        ps[j][:, :], w3v[:, :, :],
                        s2v[t % 2][:, :, b * NB:(b + 1) * NB],
                        start=first, stop=True, skip_group_check=t > 0,
                        perf_mode=DR)
                bs2 = slice(bp * NB, (bp + 2) * NB)
                if t == T - 1:
                    for j, b in enumerate((bp, bp + 1)):
                        bs = slice(b * NB, (b + 1) * NB)
                        nc.vector.tensor_scalar(outsb[:O, bs], ps[j][:O, :],
                                                32.0, None, A.is_ge)
                    nc.sync.dma_start(out=out[:, bs2], in_=outsb[:O, bs2])
                else:
                    # m = (W<32)*W with one PSUM read per op: inverse-spike
                    # scratch ns = (psum < 32), then m = ns * psum
                    for j, b in enumerate((bp, bp + 1)):
                        bs = slice(b * NB, (b + 1) * NB)
                        ns = tp.tile([128, NB], f8, tag="ns", name="ns")
                        nc.vector.tensor_scalar(ns[:O, :], ps[j][:O, :],
                                                32.0, None, A.is_lt)
                        nc.vector.scalar_tensor_tensor(
                            m3[:O, bs], ns[:O, :], 0.0, ps[j][:O, :],
                            A.bypass, A.mult)
            return emit

        def l1_pass(t, half, pends=(), slots=()):
            """One half-batch L1 pass: 4 psum groups (2h x 2b), kk inner.
            Pending L2/L3 pair closures are emitted at the given kk slots."""
            boff = half * HB
            pends = list(pends)
            slots = sorted(slots)[:len(pends)]
            ps1 = [[pp1.tile([128, NB], f32, tag="ps1", name=f"ps1_{t}_{half}_{h}_{b}")
                    for b in range(2)] for h in range(2)]
            for k in range(KT2):
                xt = xp.tile([128, 2 * HB], f8, tag="x", name="xt")
                xt3 = xt[:, :].rearrange("p (i n) -> p i n", i=2)
                nc.sync.dma_start(out=xt3, in_=x[t, k, :, :, boff:boff + HB])
                for h in range(2):
                    lhsT = w1v[:, :, k * H + h * 128: k * H + h * 128 + 128]
                    for b in range(2):
                        nc.tensor.matmul(
                            ps1[h][b][:, :], lhsT, xt3[:, :, b * NB:(b + 1) * NB],
                            start=(k == 0), stop=(k == KT2 - 1), perf_mode=DR)
                if slots and k == slots[0]:
                    slots.pop(0)
                    pends.pop(0)()
            # release all 4 banks first (w-ops), then merged spikes, then membranes
            wh = [tp.tile([128, 2 * NB], f16, tag="w", name="w") for _ in range(2)]
            for h in range(2):
                for b in range(2):
                    bs = slice(boff + b * NB, boff + (b + 1) * NB)
                    dst = wh[h][:, b * NB:(b + 1) * NB]
                    if t == 0:
                        nc.scalar.copy(dst, ps1[h][b][:, :])
                    else:
                        nc.vector.scalar_tensor_tensor(
                            dst, m1[h][:, bs], 0.5, ps1[h][b][:, :], A.mult, A.add)
            for h in range(2):
                c0 = h * BC + boff
                nc.vector.tensor_scalar(s1[t % 2][:, c0:c0 + HB], wh[h][:, :],
                                        32.0, None, A.is_ge)
            if t != T - 1:
                for h in range(2):
                    wv = wh[h][:, :]
                    nc.vector.scalar_tensor_tensor(
                        m1[h][:, boff:boff + HB], wv, 32.0, wv, A.is_lt, A.mult)
            for p in pends:  # leftovers (shouldn't happen with matched slots)
                p()

        # t=0: no carried work. t=1 passes host t=0's L2/L3; t=2 passes host
        # t=1's plus the first half of t=2's own L2 and L3 (whose s1/s2 deps
        # are ready mid-stream). The rest drains in the tail.
        l1_pass(0, 0)
        l1_pass(0, 1)
        l1_pass(1, 0,
                [l2_pair(0, 0, 0, pp23, "ps23"),
                 l2_pair(0, 1, 0, pp23, "ps23"),
                 l2_pair(0, 0, 2, pp23, "ps23")], (2, 5, 8))
        l1_pass(1, 1,
                [l2_pair(0, 1, 2, pp23, "ps23"),
                 l3_pair(0, 0, pp23, "ps23"),
                 l3_pair(0, 2, pp23, "ps23")], (2, 5, 8))
        l1_pass(2, 0,
                [l2_pair(1, 0, 0, pp23, "ps23"),
                 l2_pair(1, 1, 0, pp23, "ps23"),
                 l2_pair(1, 0, 2, pp23, "ps23")], (2, 5, 8))
        l1_pass(2, 1,
                [l2_pair(1, 1, 2, pp23, "ps23"),
                 l3_pair(1, 0, pp23, "ps23"),
                 l3_pair(1, 2, pp23, "ps23"),
                 l2_pair(2, 0, 0, pp23, "ps23"),
                 l2_pair(2, 1, 0, pp23, "ps23")], (1, 3, 5, 7, 9))
        # tail
        l3_pair(2, 0, pp23, "ps23")()
        l2_pair(2, 0, 2, pp1, "ps1")()
        l2_pair(2, 1, 2, pp1, "ps1")()
        l3_pair(2, 2, pp23, "ps23")()

    nc.compile()
    return nc


def kernel(dvs, W1, b1, W2, b2, W3, b3):
    global last_results
    import ml_dtypes
    from concourse.bass_utils import run_bass_kernel_spmd

    f8 = ml_dtypes.float8_e4m3
    use_b2 = bool(np.any(b2))
    use_b3 = bool(np.any(b3))
    key = (use_b2, use_b3)
    if key not in _compiled:
        _compiled[key] = _build(use_b2, use_b3)
    nc = _compiled[key]

    one8 = np.float32(1.0).astype(f8).view(np.uint8).item()
    # x: [B, T, D] -> fp8 [T, KT2, 128, 2, B]; pad row D=2752 carries bias (x=1)
    Xq = np.empty((T, DP, B), np.uint8)
    for t in range(T):
        Xq[t, :D, :] = np.asarray(dvs[:, t, :]).astype(f8).view(np.uint8).T
        Xq[t, D, :] = one8
        Xq[t, D + 1:, :] = 0
    Xr = Xq.reshape(T, KT2, 2, 128, B).swapaxes(2, 3)  # [T, KT2, 128, 2, B] view

    # weights scaled by 16, packed [p, i, ...] for DoubleRow
    W1p = np.zeros((DP, H), np.float32)
    W1p[:D] = W1.T * 16.0
    W1p[D] = b1 * 16.0
    w1q = W1p.astype(f8).reshape(KT2, 2, 128, H)
    w1dr = np.ascontiguousarray(
        w1q.transpose(2, 1, 0, 3).reshape(128, 2 * KT2 * H))
    w2dr = np.ascontiguousarray(
        (W2.T * 16.0).astype(f8).reshape(2, 128, H).transpose(1, 0, 2).reshape(128, 2 * H))
    W3p = np.zeros((H, 128), np.float32)
    W3p[:, :O] = W3.T * 16.0
    w3dr = np.ascontiguousarray(
        W3p.astype(f8).reshape(2, 128, 128).transpose(1, 0, 2).reshape(128, 256))

    in_maps = []
    for c in range(NCORES):
        xc = np.ascontiguousarray(Xr[:, :, :, :, c * BC:(c + 1) * BC]).view(f8)
        m = {"x": xc, "w1": w1dr, "w2": w2dr, "w3": w3dr}
        if use_b2:
            m["b2"] = (b2 * 16.0).astype(np.float16).reshape(1, H)
        if use_b3:
            m["b3"] = (b3 * 16.0).astype(np.float16).reshape(1, O)
        in_maps.append(m)

    trace = bool(os.environ.get("SNN_TRACE"))
    last_results = run_bass_kernel_spmd(nc, in_maps, core_ids=list(range(NCORES)),
                                        trace=trace)
    outv = np.empty((B, O), dtype=np.float32)
    for c in range(NCORES):
        outv[c * BC:(c + 1) * BC, :] = last_results.results[c]["out"].T
    return outv
